# revision 29
# baseline (speedup 1.0000x reference)
"""ConsistencyLoss kernel for 8 TRN2 NeuronCores (Bass/Tile).

loss = mean_b mean_{j,k} | |m1_j - m1_k| - |m2_j - m2_k| |
  m1 = per-segment means of channel-mean(input)
  m2 = per-segment means of channel-mean(bilinear_up(feature))

Sharding: data-parallel over batch B=8, one batch element per core.
Per-core pipeline (v2):
  - channel-mean of input on DVE via bf16 in-place add tree (h on partitions)
  - feature: channel-mean via ones-matmul, separable bilinear upsample via two
    small matmuls with host-built interpolation matrices
  - segment reduction, S=256=16*16 hi/lo one-hot decomposition, grouped
    cross-matrix matmuls: 8 pixel-columns share one [128,128] stationary
    (oh_hi), one N=384 moving block [A1|A2|oh_lo]x8 accumulates into a
    [128,384] PSUM tile; only the 8 diagonal [16,48] blocks are read out.
  - similarity-matrix L1 via K=2 outer-product matmuls + DVE abs/reduce
Host: casts sp to bf16 lo/hi planes, builds interp matrices, averages the
8 per-core sums.
"""

import sys

if "/opt/trn_rl_repo" not in sys.path:
    sys.path.insert(0, "/opt/trn_rl_repo")

import numpy as np
import ml_dtypes

import concourse.bacc as bacc
import concourse.mybir as mybir
import concourse.tile as tile
from concourse.bass_utils import run_bass_kernel_spmd

B, C, H, W = 8, 64, 256, 256
FH, FW = 64, 64
S = 256
N_CORES = 8

F32 = mybir.dt.float32
BF16 = mybir.dt.bfloat16
FP8 = mybir.dt.float8e4

_CACHE = {}


def _interp_matrix(out_size: int, in_size: int) -> np.ndarray:
    """R [out,in]: bilinear align_corners row-interp matrix (float32)."""
    r = np.zeros((out_size, in_size), dtype=np.float64)
    ys = np.linspace(0.0, in_size - 1.0, out_size)
    y0 = np.floor(ys).astype(np.int64)
    y1 = np.minimum(y0 + 1, in_size - 1)
    wy = ys - y0
    for o in range(out_size):
        r[o, y0[o]] += 1.0 - wy[o]
        r[o, y1[o]] += wy[o]
    return r.astype(np.float32)


def _build_nc(dbg=False):
    nc = bacc.Bacc("TRN2", target_bir_lowering=False, debug=False,
                   num_devices=N_CORES)

    LOW = 8
    HIW = 32
    MOVW = 3 * LOW
    GW = 128 // HIW
    NG = W // GW

    x = nc.dram_tensor("x", [C, H, W], F32, kind="ExternalInput").ap()
    f = nc.dram_tensor("f", [C, FH * FW], BF16, kind="ExternalInput").ap()
    lo = nc.dram_tensor("lo", [H, W], BF16, kind="ExternalInput").ap()
    ohhi_in = nc.dram_tensor("ohhi", [H, W * 32], FP8, kind="ExternalInput").ap()
    iota16 = nc.dram_tensor("iota16", [128, 16], BF16, kind="ExternalInput").ap()
    ryt = nc.dram_tensor("ryt", [FH, H], F32, kind="ExternalInput").ap()
    rxt = nc.dram_tensor("rxt", [FW, W], F32, kind="ExternalInput").ap()
    ones64 = nc.dram_tensor("ones64", [C, 1], BF16, kind="ExternalInput").ap()
    ones128 = nc.dram_tensor("ones128", [128, 1], F32, kind="ExternalInput").ap()
    onesrow = nc.dram_tensor("onesrow", [1, S], F32, kind="ExternalInput").ap()
    emat = nc.dram_tensor("emat", [128, 32], F32, kind="ExternalInput").ap()
    lh4b = nc.dram_tensor("lh4b", [4, S], F32, kind="ExternalInput").ap()
    rh4b = nc.dram_tensor("rh4b", [4, 2 * S], F32, kind="ExternalInput").ap()
    mask = nc.dram_tensor("mask", [128, GW * MOVW], F32, kind="ExternalInput").ap()
    out = nc.dram_tensor("out", [1, 1], F32, kind="ExternalOutput").ap()
    if dbg:
        dbg_acc = nc.dram_tensor("dbg_acc", [32, 24], F32, kind="ExternalOutput").ap()
        dbg_px = nc.dram_tensor("dbg_px", [128, W], F32, kind="ExternalOutput").ap()
        dbg_m1 = nc.dram_tensor("dbg_m1", [32, 8], F32, kind="ExternalOutput").ap()
        dbg_m2 = nc.dram_tensor("dbg_m2", [32, 8], F32, kind="ExternalOutput").ap()

    CCH = 16          # channels per input DMA chunk
    NCC = C // CCH

    with tile.TileContext(nc) as tc:
        with (
            tc.tile_pool(name="const", bufs=1) as const,
            tc.tile_pool(name="xin", bufs=8) as xin,
            tc.tile_pool(name="tree", bufs=1) as treep,
            tc.tile_pool(name="mov", bufs=2) as movp,
            tc.tile_pool(name="ohp", bufs=2) as ohp,
            tc.tile_pool(name="work", bufs=2) as work,
            tc.tile_pool(name="small", bufs=2) as small,
            tc.tile_pool(name="tail", bufs=1) as tailp,
            tc.tile_pool(name="ps1", bufs=1, space="PSUM") as ps1,
            tc.tile_pool(name="fmp", bufs=2, space="PSUM") as fmp,
            tc.tile_pool(name="psacc", bufs=1, space="PSUM") as psacc,
            tc.tile_pool(name="dps", bufs=1, space="PSUM") as dps,
        ):
            # ---- constants (SWDGE path; sync queue reserved for x) ----
            iota_sb = const.tile([128, 16], BF16, tag="iota")
            nc.gpsimd.dma_start(iota_sb[:], iota16[:])
            ryt_sb = const.tile([FH, H], F32, tag="ryt")
            nc.gpsimd.dma_start(ryt_sb[:], ryt[:])
            rxt_sb = const.tile([FW, W], F32, tag="rxt")
            nc.gpsimd.dma_start(rxt_sb[:], rxt[:])
            ones64_sb = const.tile([C, 1], BF16, tag="o64")
            nc.gpsimd.dma_start(ones64_sb[:], ones64[:])
            ones128_sb = const.tile([128, 1], F32, tag="o128")
            nc.gpsimd.dma_start(ones128_sb[:], ones128[:])
            emat_sb = const.tile([128, 32], F32, tag="emat")
            nc.gpsimd.dma_start(emat_sb[:], emat[:])
            mask_sb = const.tile([128, GW * MOVW], F32, tag="mask")
            nc.gpsimd.dma_start(mask_sb[:], mask[:])

            # loss-stage operand bases prefilled early
            lh4 = const.tile([4, S], F32, tag="lh4")
            nc.gpsimd.dma_start(lh4[:], lh4b[:])
            rh4 = const.tile([4, 2 * S], F32, tag="rh4")
            nc.gpsimd.dma_start(rh4[:], rh4b[:])

            # ---- feature path: channel mean -> fm [64 h', 64 w'] ----
            fsb = const.tile([C, FH * FW], BF16, tag="fsb")
            nc.gpsimd.dma_start(fsb[:], f[:])
            fmsb = const.tile([FH, FW], F32, tag="fmsb")
            for i in range(8):
                fm_ps = fmp.tile([1, 512], F32, tag="fmps")
                nc.tensor.matmul(fm_ps[:], ones64_sb[:], fsb[:, i * 512:(i + 1) * 512])
                fmpart = small.tile([1, 512], F32, tag="fmpart")
                nc.scalar.copy(fmpart[:], fm_ps[:])
                nc.sync.dma_start(fmsb[i * 8:(i + 1) * 8, :], fmpart[:])

            # ---- bilinear upsample: fmup = Ry @ fm @ Rx^T ----
            t1_ps = ps1.tile([FW, H], F32, tag="t1ps")
            nc.tensor.matmul(t1_ps[:], fmsb[:], ryt_sb[:])
            t1_sb = const.tile([FW, H], F32, tag="t1sb")
            nc.scalar.copy(t1_sb[:], t1_ps[:])

            px2bf = []
            for hb in range(2):
                up_ps = ps1.tile([128, W], F32, tag="upps")
                nc.tensor.matmul(up_ps[:], t1_sb[:, hb * 128:(hb + 1) * 128],
                                 rxt_sb[:])
                p2 = work.tile([128, W], BF16, tag=f"px2bf{hb}")
                nc.scalar.copy(p2[:], up_ps[:])
                px2bf.append(p2)

            # ---- main loop over h-blocks ----
            acc_ps = psacc.tile([128, GW * MOVW], F32, tag="acc")
            for hb in range(2):
                # input channel-sum via accumulate-DMA (CCE adds in DMA path):
                # 2 tiles x 4 overlaid 8-channel slabs -> [128, 8, 256] each,
                # then a small bf16 tree on DVE.
                parts = []
                for cc in range(NCC):
                    xt = xin.tile([128, CCH, W], F32, tag="xt")
                    nc.sync.dma_start(
                        xt[:],
                        x[cc * CCH:(cc + 1) * CCH,
                          hb * 128:(hb + 1) * 128, :].rearrange("c h w -> h c w"),
                    )
                    t8 = treep.tile([128, 8, W], BF16, tag=f"t8_{cc % 2}")
                    nc.vector.tensor_add(t8[:], xt[:, 0:8, :], xt[:, 8:16, :])
                    nc.vector.tensor_add(t8[:, 0:4, :], t8[:, 0:4, :],
                                         t8[:, 4:8, :])
                    nc.vector.tensor_add(t8[:, 0:2, :], t8[:, 0:2, :],
                                         t8[:, 2:4, :])
                    nc.vector.tensor_add(t8[:, 0:1, :], t8[:, 0:1, :],
                                         t8[:, 1:2, :])
                    if cc % 2 == 1:
                        psum_t = treep.tile([128, W], BF16, tag=f"pp{cc // 2}")
                        nc.vector.tensor_add(psum_t[:].unsqueeze(1),
                                             parts[-1][:, 0:1, :],
                                             t8[:, 0:1, :])
                        parts[-1] = psum_t
                    else:
                        parts.append(t8)
                px1 = work.tile([128, W], BF16, tag="px1bf")
                nc.vector.tensor_add(px1[:], parts[0][:], parts[1][:])
                if dbg and hb == 0:
                    pxf = tailp.tile([128, W], F32, tag="dbgpx")
                    nc.vector.tensor_copy(pxf[:], px1[:])
                    nc.sync.dma_start(dbg_px[:], pxf[:])

                # segment ids + prebuilt hi one-hot
                lo_sb = work.tile([128, W], BF16, tag="losb")
                nc.gpsimd.dma_start(lo_sb[:], lo[hb * 128:(hb + 1) * 128, :])
                ohhi = ohp.tile([128, W, HIW], FP8, tag="ohhi")
                nc.gpsimd.dma_start(
                    ohhi.rearrange("p w j -> p (w j)"),
                    ohhi_in[hb * 128:(hb + 1) * 128, :])

                # lo one-hot + A tiles in (w, j) layout, built per w-half so
                # the grouped matmuls overlap the builds
                mov = movp.tile([128, W, MOVW], BF16, tag="mov")
                WH = W // 4
                # ohlo + A2 do not depend on x -> build during the stream
                for wh in range(4):
                    ws = slice(wh * WH, (wh + 1) * WH)
                    iota_b = iota_sb[:, 0:LOW].unsqueeze(1).to_broadcast(
                        [128, WH, LOW])
                    lo_b = lo_sb[:, ws].unsqueeze(2).to_broadcast(
                        [128, WH, LOW])
                    px2_b = px2bf[hb][:, ws].unsqueeze(2).to_broadcast(
                        [128, WH, LOW])
                    nc.vector.tensor_tensor(mov[:, ws, 2 * LOW:3 * LOW],
                                            iota_b, lo_b,
                                            op=mybir.AluOpType.is_equal)
                    nc.vector.tensor_tensor(mov[:, ws, LOW:2 * LOW],
                                            mov[:, ws, 2 * LOW:3 * LOW], px2_b,
                                            op=mybir.AluOpType.mult)
                # A1 needs px1; interleave with the grouped matmuls
                for wh in range(4):
                    ws = slice(wh * WH, (wh + 1) * WH)
                    px1_b = px1[:, ws].unsqueeze(2).to_broadcast(
                        [128, WH, LOW])
                    nc.vector.tensor_tensor(mov[:, ws, 0:LOW],
                                            mov[:, ws, 2 * LOW:3 * LOW], px1_b,
                                            op=mybir.AluOpType.mult)
                    ng2 = NG // 4
                    for g in range(wh * ng2, (wh + 1) * ng2):
                        lhs = ohhi[:, g * GW:(g + 1) * GW, :].rearrange(
                            "p w j -> p (w j)")
                        rhs = mov[:, g * GW:(g + 1) * GW, :].rearrange(
                            "p w j -> p (w j)")
                        nc.tensor.matmul(
                            acc_ps[:], lhs, rhs,
                            start=(hb == 0 and g == 0),
                            stop=(hb == 1 and g == NG - 1))

            # ---- diagonal extraction: mask off-diag blocks, sum row-blocks
            #      via E^T matmul, reduce slots on free axis ----
            acc_all = tailp.tile([128, GW * MOVW], F32, tag="accall")
            nc.vector.tensor_tensor(acc_all[:], acc_ps[:], mask_sb[:],
                                    op=mybir.AluOpType.mult)
            ex_ps = ps1.tile([HIW, GW * MOVW], F32, tag="expps")
            nc.tensor.matmul(ex_ps[:], emat_sb[:], acc_all[:])
            acc_sb = small.tile([HIW, MOVW], F32, tag="accsb")
            nc.vector.tensor_reduce(
                acc_sb[:], ex_ps.rearrange("p (s j) -> p j s", s=GW),
                axis=mybir.AxisListType.X, op=mybir.AluOpType.add)
            if dbg:
                nc.sync.dma_start(dbg_acc[:], acc_sb[:])

            # ---- m1/m2 [32, 8] -> combined [32, 16] ----
            cntm = small.tile([HIW, LOW], F32, tag="cntm")
            nc.vector.tensor_scalar_max(cntm[:], acc_sb[:, 2 * LOW:3 * LOW], 0.5)
            rc2 = small.tile([HIW, LOW], F32, tag="rc2")
            nc.vector.reciprocal(rc2[:], cntm[:])
            # sum1 columns of mask are pre-scaled by 1/C, so one shared rc works
            mcomb = small.tile([HIW, 2 * LOW], F32, tag="mcomb")
            m1 = mcomb[:, 0:LOW]
            m2 = mcomb[:, LOW:2 * LOW]
            rc_b = rc2[:, :].unsqueeze(1).to_broadcast([HIW, 2, LOW])
            nc.vector.tensor_tensor(
                mcomb.rearrange("p (a b) -> p a b", a=2),
                acc_sb[:, 0:2 * LOW].rearrange("p (a b) -> p a b", a=2),
                rc_b, op=mybir.AluOpType.mult)
            if dbg:
                nc.sync.dma_start(dbg_m1[:], m1)
                nc.sync.dma_start(dbg_m2[:], m2)

            # fill m rows of the loss operands:
            # lh4 = [m1row; -1; m2row; -1], rh4 = [1|0; m1row|0; 0|1; 0|m2row]
            nc.sync.dma_start(lh4[0:1, :], m1)
            nc.sync.dma_start(lh4[2:3, :], m2)
            nc.sync.dma_start(rh4[1:2, 0:S], m1)
            nc.sync.dma_start(rh4[3:4, S:2 * S], m2)

            # ---- loss: sum_{j,k} ||m1_j-m1_k| - |m2_j-m2_k|| ----
            # one K=4 matmul per j-block -> [128, 512] = [d1 | d2]
            total = small.tile([128, 1], F32, tag="total")
            for jb in range(2):
                dc_ps = dps.tile([128, 2 * S], F32, tag="dcomb")
                nc.tensor.matmul(dc_ps[:], lh4[:, jb * 128:(jb + 1) * 128],
                                 rh4[:])
                absd = tailp.tile([128, 2 * S], F32, tag="absd")
                nc.scalar.activation(absd[:], dc_ps[:],
                                     mybir.ActivationFunctionType.Abs)
                dd = tailp.tile([128, S], F32, tag="dd")
                nc.vector.tensor_tensor(dd[:], absd[:, 0:S], absd[:, S:2 * S],
                                        op=mybir.AluOpType.subtract)
                part = small.tile([128, 1], F32, tag=f"part{jb}")
                nc.vector.tensor_reduce(
                    part[:], dd[:], axis=mybir.AxisListType.X,
                    op=mybir.AluOpType.add, apply_absolute_value=True)
                if jb == 0:
                    tot0 = part
                else:
                    nc.vector.tensor_add(total[:], tot0[:], part[:])

            loss_ps = ps1.tile([1, 1], F32, tag="t1ps")
            nc.tensor.matmul(loss_ps[:], ones128_sb[:], total[:])
            loss_sb = small.tile([1, 1], F32, tag="losssb")
            nc.vector.tensor_copy(loss_sb[:], loss_ps[:])
            nc.sync.dma_start(out[:], loss_sb[:])

    nc.compile()
    return nc


def _get_nc():
    if "nc" not in _CACHE:
        _CACHE["nc"] = _build_nc()
    return _CACHE["nc"]


def _host_inputs(input, feature, sp):
    sp32 = np.asarray(sp).astype(np.int32).reshape(B, H, W)
    lo = (sp32 & 7).astype(ml_dtypes.bfloat16)
    ohhi = ((sp32 >> 3)[..., None] ==
            np.arange(32, dtype=np.int32)).astype(ml_dtypes.float8_e4m3)
    ohhi = np.ascontiguousarray(ohhi.reshape(B, H, W * 32))
    iota16 = np.broadcast_to(
        np.arange(16, dtype=np.float32)[None, :], (128, 16)
    ).astype(ml_dtypes.bfloat16)
    iota16 = np.ascontiguousarray(iota16)
    ryt = np.ascontiguousarray(_interp_matrix(H, FH).T)   # [64, 256]
    rxt = np.ascontiguousarray(_interp_matrix(W, FW).T)   # [64, 256]
    ones64 = np.full((C, 1), 1.0 / C, dtype=ml_dtypes.bfloat16)
    ones128 = np.ones((128, 1), dtype=np.float32)
    onesrow = np.ones((1, S), dtype=np.float32)
    emat = np.zeros((128, 32), dtype=np.float32)
    for i in range(4):
        for jh in range(32):
            emat[32 * i + jh, jh] = 1.0
    mask = np.zeros((128, 96), dtype=np.float32)
    for i in range(4):
        mask[32 * i:32 * (i + 1), 24 * i:24 * (i + 1)] = 1.0
        mask[32 * i:32 * (i + 1), 24 * i:24 * i + 8] = 1.0 / C
    lh4b = np.zeros((4, 256), dtype=np.float32)
    lh4b[1, :] = -1.0
    lh4b[3, :] = -1.0
    rh4b = np.zeros((4, 512), dtype=np.float32)
    rh4b[0, 0:256] = 1.0
    rh4b[2, 256:512] = 1.0
    xf = np.ascontiguousarray(np.asarray(input, dtype=np.float32))
    ff = np.ascontiguousarray(
        np.asarray(feature, dtype=np.float32)
        .reshape(B, C, FH * FW).astype(ml_dtypes.bfloat16))
    in_maps = []
    for b in range(B):
        in_maps.append({
            "x": xf[b],
            "f": ff[b],
            "lo": np.ascontiguousarray(lo[b]),
            "ohhi": ohhi[b],
            "iota16": iota16,
            "ryt": ryt,
            "rxt": rxt,
            "ones64": ones64,
            "ones128": ones128,
            "onesrow": onesrow,
            "emat": emat,
            "mask": mask,
            "lh4b": lh4b,
            "rh4b": rh4b,
        })
    return in_maps


def _run(inputs, trace=False, **kw):
    nc = _get_nc()
    in_maps = _host_inputs(inputs["input"], inputs["feature"], inputs["sp"])
    res = run_bass_kernel_spmd(nc, in_maps, core_ids=list(range(N_CORES)),
                               trace=trace, **kw)
    sums = np.array([res.results[i]["out"][0, 0] for i in range(N_CORES)],
                    dtype=np.float64)
    loss = (sums / float(S * S)).mean()
    return np.float32(loss), res


def kernel(**inputs) -> np.ndarray:
    loss, _ = _run(inputs, trace=False)
    return np.asarray(loss, dtype=np.float32)


# revision 32
# speedup vs baseline: 1.0136x; 1.0136x over previous
"""ConsistencyLoss kernel for 8 TRN2 NeuronCores (Bass/Tile).

loss = mean_b mean_{j,k} | |m1_j - m1_k| - |m2_j - m2_k| |
  m1 = per-segment means of channel-mean(input)
  m2 = per-segment means of channel-mean(bilinear_up(feature))

Sharding: data-parallel over batch B=8, one batch element per core.
Per-core pipeline (v2):
  - channel-mean of input on DVE via bf16 in-place add tree (h on partitions)
  - feature: channel-mean via ones-matmul, separable bilinear upsample via two
    small matmuls with host-built interpolation matrices
  - segment reduction, S=256=16*16 hi/lo one-hot decomposition, grouped
    cross-matrix matmuls: 8 pixel-columns share one [128,128] stationary
    (oh_hi), one N=384 moving block [A1|A2|oh_lo]x8 accumulates into a
    [128,384] PSUM tile; only the 8 diagonal [16,48] blocks are read out.
  - similarity-matrix L1 via K=2 outer-product matmuls + DVE abs/reduce
Host: casts sp to bf16 lo/hi planes, builds interp matrices, averages the
8 per-core sums.
"""

import sys

if "/opt/trn_rl_repo" not in sys.path:
    sys.path.insert(0, "/opt/trn_rl_repo")

import numpy as np
import ml_dtypes

import concourse.bacc as bacc
import concourse.mybir as mybir
import concourse.tile as tile
from concourse.bass_utils import run_bass_kernel_spmd

B, C, H, W = 8, 64, 256, 256
FH, FW = 64, 64
S = 256
N_CORES = 8

F32 = mybir.dt.float32
BF16 = mybir.dt.bfloat16
FP8 = mybir.dt.float8e4

_CACHE = {}


def _interp_matrix(out_size: int, in_size: int) -> np.ndarray:
    """R [out,in]: bilinear align_corners row-interp matrix (float32)."""
    r = np.zeros((out_size, in_size), dtype=np.float64)
    ys = np.linspace(0.0, in_size - 1.0, out_size)
    y0 = np.floor(ys).astype(np.int64)
    y1 = np.minimum(y0 + 1, in_size - 1)
    wy = ys - y0
    for o in range(out_size):
        r[o, y0[o]] += 1.0 - wy[o]
        r[o, y1[o]] += wy[o]
    return r.astype(np.float32)


def _build_nc(dbg=False):
    nc = bacc.Bacc("TRN2", target_bir_lowering=False, debug=False,
                   num_devices=N_CORES)

    LOW = 8
    HIW = 32
    MOVW = 3 * LOW
    GW = 128 // HIW
    NG = W // GW

    x = nc.dram_tensor("x", [C, H, W], F32, kind="ExternalInput").ap()
    f = nc.dram_tensor("f", [C, FH * FW], BF16, kind="ExternalInput").ap()
    lo = nc.dram_tensor("lo", [H, W], BF16, kind="ExternalInput").ap()
    ohhi_in = nc.dram_tensor("ohhi", [H, W * 32], FP8, kind="ExternalInput").ap()
    iota16 = nc.dram_tensor("iota16", [128, 16], BF16, kind="ExternalInput").ap()
    ryt = nc.dram_tensor("ryt", [FH, H], F32, kind="ExternalInput").ap()
    rxt = nc.dram_tensor("rxt", [FW, W], F32, kind="ExternalInput").ap()
    ones64 = nc.dram_tensor("ones64", [C, 1], BF16, kind="ExternalInput").ap()
    ones128 = nc.dram_tensor("ones128", [128, 1], F32, kind="ExternalInput").ap()
    onesrow = nc.dram_tensor("onesrow", [1, S], F32, kind="ExternalInput").ap()
    emat = nc.dram_tensor("emat", [128, 32], F32, kind="ExternalInput").ap()
    lh4b = nc.dram_tensor("lh4b", [4, S], F32, kind="ExternalInput").ap()
    rh4b = nc.dram_tensor("rh4b", [4, 2 * S], F32, kind="ExternalInput").ap()
    mask = nc.dram_tensor("mask", [128, GW * MOVW], F32, kind="ExternalInput").ap()
    out = nc.dram_tensor("out", [1, 1], F32, kind="ExternalOutput").ap()
    if dbg:
        dbg_acc = nc.dram_tensor("dbg_acc", [32, 24], F32, kind="ExternalOutput").ap()
        dbg_px = nc.dram_tensor("dbg_px", [128, W], F32, kind="ExternalOutput").ap()
        dbg_m1 = nc.dram_tensor("dbg_m1", [32, 8], F32, kind="ExternalOutput").ap()
        dbg_m2 = nc.dram_tensor("dbg_m2", [32, 8], F32, kind="ExternalOutput").ap()

    CCH = 16          # channels per input DMA chunk
    NCC = C // CCH

    with tile.TileContext(nc) as tc:
        with (
            tc.tile_pool(name="const", bufs=1) as const,
            tc.tile_pool(name="xin", bufs=8) as xin,
            tc.tile_pool(name="tree", bufs=1) as treep,
            tc.tile_pool(name="mov", bufs=2) as movp,
            tc.tile_pool(name="work", bufs=2) as work,
            tc.tile_pool(name="small", bufs=2) as small,
            tc.tile_pool(name="tail", bufs=1) as tailp,
            tc.tile_pool(name="ps1", bufs=1, space="PSUM") as ps1,
            tc.tile_pool(name="fmp", bufs=2, space="PSUM") as fmp,
            tc.tile_pool(name="psacc", bufs=1, space="PSUM") as psacc,
            tc.tile_pool(name="dps", bufs=1, space="PSUM") as dps,
        ):
            # ---- constants (SWDGE path; sync queue reserved for x) ----
            iota_sb = const.tile([128, 16], BF16, tag="iota")
            nc.gpsimd.dma_start(iota_sb[:], iota16[:])
            ryt_sb = const.tile([FH, H], F32, tag="ryt")
            nc.gpsimd.dma_start(ryt_sb[:], ryt[:])
            rxt_sb = const.tile([FW, W], F32, tag="rxt")
            nc.gpsimd.dma_start(rxt_sb[:], rxt[:])
            ones64_sb = const.tile([C, 1], BF16, tag="o64")
            nc.gpsimd.dma_start(ones64_sb[:], ones64[:])
            ones128_sb = const.tile([128, 1], F32, tag="o128")
            nc.gpsimd.dma_start(ones128_sb[:], ones128[:])
            emat_sb = const.tile([128, 32], F32, tag="emat")
            nc.gpsimd.dma_start(emat_sb[:], emat[:])
            mask_sb = const.tile([128, GW * MOVW], F32, tag="mask")
            nc.gpsimd.dma_start(mask_sb[:], mask[:])

            # loss-stage operand bases prefilled early
            lh4 = const.tile([4, S], F32, tag="lh4")
            nc.gpsimd.dma_start(lh4[:], lh4b[:])
            rh4 = const.tile([4, 2 * S], F32, tag="rh4")
            nc.gpsimd.dma_start(rh4[:], rh4b[:])

            # ---- feature path: channel mean -> fm [64 h', 64 w'] ----
            fsb = const.tile([C, FH * FW], BF16, tag="fsb")
            nc.gpsimd.dma_start(fsb[:], f[:])
            fmsb = const.tile([FH, FW], F32, tag="fmsb")
            for i in range(8):
                fm_ps = fmp.tile([1, 512], F32, tag="fmps")
                nc.tensor.matmul(fm_ps[:], ones64_sb[:], fsb[:, i * 512:(i + 1) * 512])
                fmpart = small.tile([1, 512], F32, tag="fmpart")
                nc.scalar.copy(fmpart[:], fm_ps[:])
                nc.sync.dma_start(fmsb[i * 8:(i + 1) * 8, :], fmpart[:])

            # ---- bilinear upsample: fmup = Ry @ fm @ Rx^T ----
            t1_ps = ps1.tile([FW, H], F32, tag="t1ps")
            nc.tensor.matmul(t1_ps[:], fmsb[:], ryt_sb[:])
            t1_sb = const.tile([FW, H], F32, tag="t1sb")
            nc.scalar.copy(t1_sb[:], t1_ps[:])

            px2bf = []
            for hb in range(2):
                up_ps = ps1.tile([128, W], F32, tag="upps")
                nc.tensor.matmul(up_ps[:], t1_sb[:, hb * 128:(hb + 1) * 128],
                                 rxt_sb[:])
                p2 = work.tile([128, W], BF16, tag=f"px2bf{hb}")
                nc.scalar.copy(p2[:], up_ps[:])
                px2bf.append(p2)

            # ---- main loop over h-blocks ----
            lo_sbs, ohhi_sbs = [], []
            for hb in range(2):
                lo_t = const.tile([128, W], BF16, tag=f"losb{hb}")
                nc.gpsimd.dma_start(lo_t[:], lo[hb * 128:(hb + 1) * 128, :])
                lo_sbs.append(lo_t)
                oh_t = const.tile([128, W, HIW], FP8, tag=f"ohhi{hb}")
                nc.gpsimd.dma_start(
                    oh_t.rearrange("p w j -> p (w j)"),
                    ohhi_in[hb * 128:(hb + 1) * 128, :])
                ohhi_sbs.append(oh_t)
            acc_ps = psacc.tile([128, GW * MOVW], F32, tag="acc")
            for hb in range(2):
                # input channel-sum via accumulate-DMA (CCE adds in DMA path):
                # 2 tiles x 4 overlaid 8-channel slabs -> [128, 8, 256] each,
                # then a small bf16 tree on DVE.
                parts = []
                for cc in range(NCC):
                    xt = xin.tile([128, CCH, W], F32, tag="xt")
                    nc.sync.dma_start(
                        xt[:],
                        x[cc * CCH:(cc + 1) * CCH,
                          hb * 128:(hb + 1) * 128, :].rearrange("c h w -> h c w"),
                    )
                    t8 = treep.tile([128, 8, W], BF16, tag=f"t8_{cc % 2}")
                    nc.vector.tensor_add(t8[:], xt[:, 0:8, :], xt[:, 8:16, :])
                    nc.vector.tensor_add(t8[:, 0:4, :], t8[:, 0:4, :],
                                         t8[:, 4:8, :])
                    nc.vector.tensor_add(t8[:, 0:2, :], t8[:, 0:2, :],
                                         t8[:, 2:4, :])
                    nc.vector.tensor_add(t8[:, 0:1, :], t8[:, 0:1, :],
                                         t8[:, 1:2, :])
                    if cc % 2 == 1:
                        psum_t = treep.tile([128, W], BF16, tag=f"pp{cc // 2}")
                        nc.vector.tensor_add(psum_t[:].unsqueeze(1),
                                             parts[-1][:, 0:1, :],
                                             t8[:, 0:1, :])
                        parts[-1] = psum_t
                    else:
                        parts.append(t8)
                px1 = work.tile([128, W], BF16, tag="px1bf")
                nc.vector.tensor_add(px1[:], parts[0][:], parts[1][:])
                if dbg and hb == 0:
                    pxf = tailp.tile([128, W], F32, tag="dbgpx")
                    nc.vector.tensor_copy(pxf[:], px1[:])
                    nc.sync.dma_start(dbg_px[:], pxf[:])

                lo_sb = lo_sbs[hb]
                ohhi = ohhi_sbs[hb]

                # lo one-hot + A tiles in (w, j) layout, built per w-half so
                # the grouped matmuls overlap the builds
                mov = movp.tile([128, W, MOVW], BF16, tag="mov")
                WH = W // 4
                # ohlo + A2 do not depend on x -> build during the stream
                for wh in range(4):
                    ws = slice(wh * WH, (wh + 1) * WH)
                    iota_b = iota_sb[:, 0:LOW].unsqueeze(1).to_broadcast(
                        [128, WH, LOW])
                    lo_b = lo_sb[:, ws].unsqueeze(2).to_broadcast(
                        [128, WH, LOW])
                    px2_b = px2bf[hb][:, ws].unsqueeze(2).to_broadcast(
                        [128, WH, LOW])
                    nc.vector.tensor_tensor(mov[:, ws, 2 * LOW:3 * LOW],
                                            iota_b, lo_b,
                                            op=mybir.AluOpType.is_equal)
                    nc.vector.tensor_tensor(mov[:, ws, LOW:2 * LOW],
                                            mov[:, ws, 2 * LOW:3 * LOW], px2_b,
                                            op=mybir.AluOpType.mult)
                # A1 needs px1; interleave with the grouped matmuls
                for wh in range(4):
                    ws = slice(wh * WH, (wh + 1) * WH)
                    px1_b = px1[:, ws].unsqueeze(2).to_broadcast(
                        [128, WH, LOW])
                    nc.vector.tensor_tensor(mov[:, ws, 0:LOW],
                                            mov[:, ws, 2 * LOW:3 * LOW], px1_b,
                                            op=mybir.AluOpType.mult)
                    ng2 = NG // 4
                    for g in range(wh * ng2, (wh + 1) * ng2):
                        lhs = ohhi[:, g * GW:(g + 1) * GW, :].rearrange(
                            "p w j -> p (w j)")
                        rhs = mov[:, g * GW:(g + 1) * GW, :].rearrange(
                            "p w j -> p (w j)")
                        nc.tensor.matmul(
                            acc_ps[:], lhs, rhs,
                            start=(hb == 0 and g == 0),
                            stop=(hb == 1 and g == NG - 1))

            # ---- diagonal extraction: mask off-diag blocks, sum row-blocks
            #      via E^T matmul, reduce slots on free axis ----
            acc_all = tailp.tile([128, GW * MOVW], F32, tag="accall")
            nc.vector.tensor_tensor(acc_all[:], acc_ps[:], mask_sb[:],
                                    op=mybir.AluOpType.mult)
            ex_ps = ps1.tile([HIW, GW * MOVW], F32, tag="expps")
            nc.tensor.matmul(ex_ps[:], emat_sb[:], acc_all[:])
            acc_sb = small.tile([HIW, MOVW], F32, tag="accsb")
            nc.vector.tensor_reduce(
                acc_sb[:], ex_ps.rearrange("p (s j) -> p j s", s=GW),
                axis=mybir.AxisListType.X, op=mybir.AluOpType.add)
            if dbg:
                nc.sync.dma_start(dbg_acc[:], acc_sb[:])

            # ---- m1/m2 [32, 8] -> combined [32, 16] ----
            cntm = small.tile([HIW, LOW], F32, tag="cntm")
            nc.vector.tensor_scalar_max(cntm[:], acc_sb[:, 2 * LOW:3 * LOW], 0.5)
            rc2 = small.tile([HIW, LOW], F32, tag="rc2")
            nc.vector.reciprocal(rc2[:], cntm[:])
            # sum1 columns of mask are pre-scaled by 1/C, so one shared rc works
            mcomb = small.tile([HIW, 2 * LOW], F32, tag="mcomb")
            m1 = mcomb[:, 0:LOW]
            m2 = mcomb[:, LOW:2 * LOW]
            rc_b = rc2[:, :].unsqueeze(1).to_broadcast([HIW, 2, LOW])
            nc.vector.tensor_tensor(
                mcomb.rearrange("p (a b) -> p a b", a=2),
                acc_sb[:, 0:2 * LOW].rearrange("p (a b) -> p a b", a=2),
                rc_b, op=mybir.AluOpType.mult)
            if dbg:
                nc.sync.dma_start(dbg_m1[:], m1)
                nc.sync.dma_start(dbg_m2[:], m2)

            # fill m rows of the loss operands:
            # lh4 = [m1row; -1; m2row; -1], rh4 = [1|0; m1row|0; 0|1; 0|m2row]
            nc.sync.dma_start(lh4[0:1, :], m1)
            nc.scalar.dma_start(rh4[1:2, 0:S], m1)
            nc.sync.dma_start(lh4[2:3, :], m2)
            nc.scalar.dma_start(rh4[3:4, S:2 * S], m2)

            # ---- loss: sum_{j,k} ||m1_j-m1_k| - |m2_j-m2_k|| ----
            # one K=4 matmul per j-block -> [128, 512] = [d1 | d2]
            total = small.tile([128, 1], F32, tag="total")
            for jb in range(2):
                dc_ps = dps.tile([128, 2 * S], F32, tag="dcomb")
                nc.tensor.matmul(dc_ps[:], lh4[:, jb * 128:(jb + 1) * 128],
                                 rh4[:])
                absd = tailp.tile([128, 2 * S], F32, tag="absd")
                nc.scalar.activation(absd[:], dc_ps[:],
                                     mybir.ActivationFunctionType.Abs)
                dd = tailp.tile([128, S], F32, tag="dd")
                nc.vector.tensor_tensor(dd[:], absd[:, 0:S], absd[:, S:2 * S],
                                        op=mybir.AluOpType.subtract)
                part = small.tile([128, 1], F32, tag=f"part{jb}")
                nc.vector.tensor_reduce(
                    part[:], dd[:], axis=mybir.AxisListType.X,
                    op=mybir.AluOpType.add, apply_absolute_value=True)
                if jb == 0:
                    tot0 = part
                else:
                    nc.vector.tensor_add(total[:], tot0[:], part[:])

            loss_ps = ps1.tile([1, 1], F32, tag="t1ps")
            nc.tensor.matmul(loss_ps[:], ones128_sb[:], total[:])
            loss_sb = small.tile([1, 1], F32, tag="losssb")
            nc.vector.tensor_copy(loss_sb[:], loss_ps[:])
            nc.sync.dma_start(out[:], loss_sb[:])

    nc.compile()
    return nc


def _get_nc():
    if "nc" not in _CACHE:
        _CACHE["nc"] = _build_nc()
    return _CACHE["nc"]


def _host_inputs(input, feature, sp):
    sp32 = np.asarray(sp).astype(np.int32).reshape(B, H, W)
    lo = (sp32 & 7).astype(ml_dtypes.bfloat16)
    ohhi = ((sp32 >> 3)[..., None] ==
            np.arange(32, dtype=np.int32)).astype(ml_dtypes.float8_e4m3)
    ohhi = np.ascontiguousarray(ohhi.reshape(B, H, W * 32))
    iota16 = np.broadcast_to(
        np.arange(16, dtype=np.float32)[None, :], (128, 16)
    ).astype(ml_dtypes.bfloat16)
    iota16 = np.ascontiguousarray(iota16)
    ryt = np.ascontiguousarray(_interp_matrix(H, FH).T)   # [64, 256]
    rxt = np.ascontiguousarray(_interp_matrix(W, FW).T)   # [64, 256]
    ones64 = np.full((C, 1), 1.0 / C, dtype=ml_dtypes.bfloat16)
    ones128 = np.ones((128, 1), dtype=np.float32)
    onesrow = np.ones((1, S), dtype=np.float32)
    emat = np.zeros((128, 32), dtype=np.float32)
    for i in range(4):
        for jh in range(32):
            emat[32 * i + jh, jh] = 1.0
    mask = np.zeros((128, 96), dtype=np.float32)
    for i in range(4):
        mask[32 * i:32 * (i + 1), 24 * i:24 * (i + 1)] = 1.0
        mask[32 * i:32 * (i + 1), 24 * i:24 * i + 8] = 1.0 / C
    lh4b = np.zeros((4, 256), dtype=np.float32)
    lh4b[1, :] = -1.0
    lh4b[3, :] = -1.0
    rh4b = np.zeros((4, 512), dtype=np.float32)
    rh4b[0, 0:256] = 1.0
    rh4b[2, 256:512] = 1.0
    xf = np.ascontiguousarray(np.asarray(input, dtype=np.float32))
    ff = np.ascontiguousarray(
        np.asarray(feature, dtype=np.float32)
        .reshape(B, C, FH * FW).astype(ml_dtypes.bfloat16))
    in_maps = []
    for b in range(B):
        in_maps.append({
            "x": xf[b],
            "f": ff[b],
            "lo": np.ascontiguousarray(lo[b]),
            "ohhi": ohhi[b],
            "iota16": iota16,
            "ryt": ryt,
            "rxt": rxt,
            "ones64": ones64,
            "ones128": ones128,
            "onesrow": onesrow,
            "emat": emat,
            "mask": mask,
            "lh4b": lh4b,
            "rh4b": rh4b,
        })
    return in_maps


def _run(inputs, trace=False, **kw):
    nc = _get_nc()
    in_maps = _host_inputs(inputs["input"], inputs["feature"], inputs["sp"])
    res = run_bass_kernel_spmd(nc, in_maps, core_ids=list(range(N_CORES)),
                               trace=trace, **kw)
    sums = np.array([res.results[i]["out"][0, 0] for i in range(N_CORES)],
                    dtype=np.float64)
    loss = (sums / float(S * S)).mean()
    return np.float32(loss), res


def kernel(**inputs) -> np.ndarray:
    loss, _ = _run(inputs, trace=False)
    return np.asarray(loss, dtype=np.float32)


# revision 33
# speedup vs baseline: 1.0223x; 1.0085x over previous
"""ConsistencyLoss kernel for 8 TRN2 NeuronCores (Bass/Tile).

loss = mean_b mean_{j,k} | |m1_j - m1_k| - |m2_j - m2_k| |
  m1 = per-segment means of channel-mean(input)       [B,64,256,256]
  m2 = per-segment means of channel-mean(bilinear_up(feature)) [B,64,64,64]

Sharding: data-parallel over batch B=8, one batch element per core; the 8
per-core loss sums are averaged on the host (the "all-reduce" of the hint).

Per-core pipeline:
  - input channel-sum on DVE: 16-channel DMA chunks ([128h, 16c, 256w],
    1 KiB descriptors, h on partitions) reduced by a bf16 in-place add tree
    (level 1 fp32->bf16, then 2x-mode bf16 adds)
  - feature path: channel-mean via ones/64-stationary matmuls, separable
    align_corners bilinear upsample as two small matmuls with host-built
    interpolation matrices (T1 = fm @ RyT, fmup = T1^T-slices @ RxT)
  - segment reduction (S=256 = 32hi x 8lo one-hot decomposition):
    oh_hi one-hots ship from host as fp8 (exact 0/1); oh_lo + A1/A2 built
    on DVE in (w, j) layout; grouped cross-matrix matmuls - 4 pixel-columns
    share one [128,128] fp8 stationary, one N=96 bf16 moving block
    [A1|A2|oh_lo]x4 accumulates into a [128,96] PSUM tile across 128 groups;
    diagonal [32,24] blocks are extracted via host-mask multiply + E^T
    matmul + one reduce (mask also folds in the 1/64 channel scale)
  - counts come from the oh_lo columns; m = sum * recip(max(cnt, .5))
    reproduces the reference's where(cnt>0, sum/max(cnt,1), 0)
  - similarity L1: one K=4 outer-product matmul per 128-row block yields
    [d1|d2] = [m_j - m_k] for both m's; ACT abs, DVE subtract and
    abs-reduce, final ones-matmul -> scalar sum per core
Host prep (cheap O(HW) int ops, part of sharding): sp -> bf16 lo plane and
fp8 hi one-hot, interpolation matrices, small constant tables.
"""

import sys

if "/opt/trn_rl_repo" not in sys.path:
    sys.path.insert(0, "/opt/trn_rl_repo")

import numpy as np
import ml_dtypes

import concourse.bacc as bacc
import concourse.mybir as mybir
import concourse.tile as tile
from concourse.bass_utils import run_bass_kernel_spmd

B, C, H, W = 8, 64, 256, 256
FH, FW = 64, 64
S = 256
N_CORES = 8

F32 = mybir.dt.float32
BF16 = mybir.dt.bfloat16
FP8 = mybir.dt.float8e4

_CACHE = {}


def _interp_matrix(out_size: int, in_size: int) -> np.ndarray:
    """R [out,in]: bilinear align_corners row-interp matrix (float32)."""
    r = np.zeros((out_size, in_size), dtype=np.float64)
    ys = np.linspace(0.0, in_size - 1.0, out_size)
    y0 = np.floor(ys).astype(np.int64)
    y1 = np.minimum(y0 + 1, in_size - 1)
    wy = ys - y0
    for o in range(out_size):
        r[o, y0[o]] += 1.0 - wy[o]
        r[o, y1[o]] += wy[o]
    return r.astype(np.float32)


def _build_nc(dbg=False):
    nc = bacc.Bacc("TRN2", target_bir_lowering=False, debug=False,
                   num_devices=N_CORES)

    LOW = 8
    HIW = 32
    MOVW = 3 * LOW
    GW = 128 // HIW
    NG = W // GW

    x = nc.dram_tensor("x", [C, H, W], F32, kind="ExternalInput").ap()
    f = nc.dram_tensor("f", [C, FH * FW], BF16, kind="ExternalInput").ap()
    lo = nc.dram_tensor("lo", [H, W], BF16, kind="ExternalInput").ap()
    ohhi_in = nc.dram_tensor("ohhi", [H, W * 32], FP8, kind="ExternalInput").ap()
    iota16 = nc.dram_tensor("iota16", [128, 16], BF16, kind="ExternalInput").ap()
    ryt = nc.dram_tensor("ryt", [FH, H], F32, kind="ExternalInput").ap()
    rxt = nc.dram_tensor("rxt", [FW, W], F32, kind="ExternalInput").ap()
    ones64 = nc.dram_tensor("ones64", [C, 1], BF16, kind="ExternalInput").ap()
    ones128 = nc.dram_tensor("ones128", [128, 1], F32, kind="ExternalInput").ap()
    onesrow = nc.dram_tensor("onesrow", [1, S], F32, kind="ExternalInput").ap()
    emat = nc.dram_tensor("emat", [128, 32], F32, kind="ExternalInput").ap()
    lh4b = nc.dram_tensor("lh4b", [4, S], F32, kind="ExternalInput").ap()
    rh4b = nc.dram_tensor("rh4b", [4, 2 * S], F32, kind="ExternalInput").ap()
    mask = nc.dram_tensor("mask", [128, GW * MOVW], F32, kind="ExternalInput").ap()
    out = nc.dram_tensor("out", [1, 1], F32, kind="ExternalOutput").ap()
    if dbg:
        dbg_acc = nc.dram_tensor("dbg_acc", [32, 24], F32, kind="ExternalOutput").ap()
        dbg_px = nc.dram_tensor("dbg_px", [128, W], F32, kind="ExternalOutput").ap()
        dbg_m1 = nc.dram_tensor("dbg_m1", [32, 8], F32, kind="ExternalOutput").ap()
        dbg_m2 = nc.dram_tensor("dbg_m2", [32, 8], F32, kind="ExternalOutput").ap()

    CCH = 16          # channels per input DMA chunk
    NCC = C // CCH

    with tile.TileContext(nc) as tc:
        with (
            tc.tile_pool(name="const", bufs=1) as const,
            tc.tile_pool(name="xin", bufs=8) as xin,
            tc.tile_pool(name="tree", bufs=1) as treep,
            tc.tile_pool(name="mov", bufs=2) as movp,
            tc.tile_pool(name="work", bufs=2) as work,
            tc.tile_pool(name="small", bufs=2) as small,
            tc.tile_pool(name="tail", bufs=1) as tailp,
            tc.tile_pool(name="ps1", bufs=1, space="PSUM") as ps1,
            tc.tile_pool(name="fmp", bufs=2, space="PSUM") as fmp,
            tc.tile_pool(name="psacc", bufs=1, space="PSUM") as psacc,
            tc.tile_pool(name="dps", bufs=1, space="PSUM") as dps,
        ):
            # ---- constants (SWDGE path; sync queue reserved for x) ----
            iota_sb = const.tile([128, 16], BF16, tag="iota")
            nc.gpsimd.dma_start(iota_sb[:], iota16[:])
            ryt_sb = const.tile([FH, H], F32, tag="ryt")
            nc.gpsimd.dma_start(ryt_sb[:], ryt[:])
            rxt_sb = const.tile([FW, W], F32, tag="rxt")
            nc.gpsimd.dma_start(rxt_sb[:], rxt[:])
            ones64_sb = const.tile([C, 1], BF16, tag="o64")
            nc.gpsimd.dma_start(ones64_sb[:], ones64[:])
            ones128_sb = const.tile([128, 1], F32, tag="o128")
            nc.gpsimd.dma_start(ones128_sb[:], ones128[:])
            emat_sb = const.tile([128, 32], F32, tag="emat")
            nc.gpsimd.dma_start(emat_sb[:], emat[:])
            mask_sb = const.tile([128, GW * MOVW], F32, tag="mask")
            nc.gpsimd.dma_start(mask_sb[:], mask[:])

            # loss-stage operand bases prefilled early
            lh4 = const.tile([4, S], F32, tag="lh4")
            nc.gpsimd.dma_start(lh4[:], lh4b[:])
            rh4 = const.tile([4, 2 * S], F32, tag="rh4")
            nc.gpsimd.dma_start(rh4[:], rh4b[:])

            # ---- feature path: channel mean -> fm [64 h', 64 w'] ----
            fsb = const.tile([C, FH * FW], BF16, tag="fsb")
            nc.gpsimd.dma_start(fsb[:], f[:])
            fmsb = const.tile([FH, FW], F32, tag="fmsb")
            for i in range(8):
                fm_ps = fmp.tile([1, 512], F32, tag="fmps")
                nc.tensor.matmul(fm_ps[:], ones64_sb[:], fsb[:, i * 512:(i + 1) * 512])
                fmpart = small.tile([1, 512], F32, tag="fmpart")
                nc.scalar.copy(fmpart[:], fm_ps[:])
                nc.sync.dma_start(fmsb[i * 8:(i + 1) * 8, :], fmpart[:])

            # ---- bilinear upsample: fmup = Ry @ fm @ Rx^T ----
            t1_ps = ps1.tile([FW, H], F32, tag="t1ps")
            nc.tensor.matmul(t1_ps[:], fmsb[:], ryt_sb[:])
            t1_sb = const.tile([FW, H], F32, tag="t1sb")
            nc.scalar.copy(t1_sb[:], t1_ps[:])

            px2bf = []
            for hb in range(2):
                up_ps = ps1.tile([128, W], F32, tag="upps")
                nc.tensor.matmul(up_ps[:], t1_sb[:, hb * 128:(hb + 1) * 128],
                                 rxt_sb[:])
                p2 = work.tile([128, W], BF16, tag=f"px2bf{hb}")
                nc.scalar.copy(p2[:], up_ps[:])
                px2bf.append(p2)

            # ---- main loop over h-blocks ----
            lo_sbs, ohhi_sbs = [], []
            for hb in range(2):
                lo_t = const.tile([128, W], BF16, tag=f"losb{hb}")
                nc.gpsimd.dma_start(lo_t[:], lo[hb * 128:(hb + 1) * 128, :])
                lo_sbs.append(lo_t)
                oh_t = const.tile([128, W, HIW], FP8, tag=f"ohhi{hb}")
                nc.gpsimd.dma_start(
                    oh_t.rearrange("p w j -> p (w j)"),
                    ohhi_in[hb * 128:(hb + 1) * 128, :])
                ohhi_sbs.append(oh_t)
            acc_ps = psacc.tile([128, GW * MOVW], F32, tag="acc")
            for hb in range(2):
                # input channel-sum via accumulate-DMA (CCE adds in DMA path):
                # 2 tiles x 4 overlaid 8-channel slabs -> [128, 8, 256] each,
                # then a small bf16 tree on DVE.
                parts = []
                for cc in range(NCC):
                    xt = xin.tile([128, CCH, W], F32, tag="xt")
                    nc.sync.dma_start(
                        xt[:],
                        x[cc * CCH:(cc + 1) * CCH,
                          hb * 128:(hb + 1) * 128, :].rearrange("c h w -> h c w"),
                    )
                    t8 = treep.tile([128, 8, W], BF16, tag=f"t8_{cc % 2}")
                    nc.vector.tensor_add(t8[:], xt[:, 0:8, :], xt[:, 8:16, :])
                    nc.vector.tensor_add(t8[:, 0:4, :], t8[:, 0:4, :],
                                         t8[:, 4:8, :])
                    nc.vector.tensor_add(t8[:, 0:2, :], t8[:, 0:2, :],
                                         t8[:, 2:4, :])
                    nc.vector.tensor_add(t8[:, 0:1, :], t8[:, 0:1, :],
                                         t8[:, 1:2, :])
                    if cc % 2 == 1:
                        psum_t = treep.tile([128, W], BF16, tag=f"pp{cc // 2}")
                        nc.vector.tensor_add(psum_t[:].unsqueeze(1),
                                             parts[-1][:, 0:1, :],
                                             t8[:, 0:1, :])
                        parts[-1] = psum_t
                    else:
                        parts.append(t8)
                px1 = work.tile([128, W], BF16, tag="px1bf")
                nc.vector.tensor_add(px1[:], parts[0][:], parts[1][:])
                if dbg and hb == 0:
                    pxf = tailp.tile([128, W], F32, tag="dbgpx")
                    nc.vector.tensor_copy(pxf[:], px1[:])
                    nc.sync.dma_start(dbg_px[:], pxf[:])

                lo_sb = lo_sbs[hb]
                ohhi = ohhi_sbs[hb]

                # lo one-hot + A tiles in (w, j) layout, built per w-half so
                # the grouped matmuls overlap the builds
                mov = movp.tile([128, W, MOVW], BF16, tag="mov")
                WH = W // 4
                # ohlo + A2 do not depend on x -> build during the stream
                for wh in range(4):
                    ws = slice(wh * WH, (wh + 1) * WH)
                    iota_b = iota_sb[:, 0:LOW].unsqueeze(1).to_broadcast(
                        [128, WH, LOW])
                    lo_b = lo_sb[:, ws].unsqueeze(2).to_broadcast(
                        [128, WH, LOW])
                    px2_b = px2bf[hb][:, ws].unsqueeze(2).to_broadcast(
                        [128, WH, LOW])
                    nc.vector.tensor_tensor(mov[:, ws, 2 * LOW:3 * LOW],
                                            iota_b, lo_b,
                                            op=mybir.AluOpType.is_equal)
                    nc.vector.tensor_tensor(mov[:, ws, LOW:2 * LOW],
                                            mov[:, ws, 2 * LOW:3 * LOW], px2_b,
                                            op=mybir.AluOpType.mult)
                # A1 needs px1; interleave with the grouped matmuls
                for wh in range(4):
                    ws = slice(wh * WH, (wh + 1) * WH)
                    px1_b = px1[:, ws].unsqueeze(2).to_broadcast(
                        [128, WH, LOW])
                    nc.vector.tensor_tensor(mov[:, ws, 0:LOW],
                                            mov[:, ws, 2 * LOW:3 * LOW], px1_b,
                                            op=mybir.AluOpType.mult)
                    ng2 = NG // 4
                    for g in range(wh * ng2, (wh + 1) * ng2):
                        lhs = ohhi[:, g * GW:(g + 1) * GW, :].rearrange(
                            "p w j -> p (w j)")
                        rhs = mov[:, g * GW:(g + 1) * GW, :].rearrange(
                            "p w j -> p (w j)")
                        nc.tensor.matmul(
                            acc_ps[:], lhs, rhs,
                            start=(hb == 0 and g == 0),
                            stop=(hb == 1 and g == NG - 1))

            # ---- diagonal extraction: mask off-diag blocks, sum row-blocks
            #      via E^T matmul, reduce slots on free axis ----
            acc_all = tailp.tile([128, GW * MOVW], F32, tag="accall")
            nc.vector.tensor_tensor(acc_all[:], acc_ps[:], mask_sb[:],
                                    op=mybir.AluOpType.mult)
            ex_ps = ps1.tile([HIW, GW * MOVW], F32, tag="expps")
            nc.tensor.matmul(ex_ps[:], emat_sb[:], acc_all[:])
            acc_sb = small.tile([HIW, MOVW], F32, tag="accsb")
            nc.vector.tensor_reduce(
                acc_sb[:], ex_ps.rearrange("p (s j) -> p j s", s=GW),
                axis=mybir.AxisListType.X, op=mybir.AluOpType.add)
            if dbg:
                nc.sync.dma_start(dbg_acc[:], acc_sb[:])

            # ---- m1/m2 [32, 8] -> combined [32, 16] ----
            cntm = small.tile([HIW, LOW], F32, tag="cntm")
            nc.vector.tensor_scalar_max(cntm[:], acc_sb[:, 2 * LOW:3 * LOW], 0.5)
            rc2 = small.tile([HIW, LOW], F32, tag="rc2")
            nc.vector.reciprocal(rc2[:], cntm[:])
            # sum1 columns of mask are pre-scaled by 1/C, so one shared rc works
            mcomb = small.tile([HIW, 2 * LOW], F32, tag="mcomb")
            m1 = mcomb[:, 0:LOW]
            m2 = mcomb[:, LOW:2 * LOW]
            rc_b = rc2[:, :].unsqueeze(1).to_broadcast([HIW, 2, LOW])
            nc.vector.tensor_tensor(
                mcomb.rearrange("p (a b) -> p a b", a=2),
                acc_sb[:, 0:2 * LOW].rearrange("p (a b) -> p a b", a=2),
                rc_b, op=mybir.AluOpType.mult)
            if dbg:
                nc.sync.dma_start(dbg_m1[:], m1)
                nc.sync.dma_start(dbg_m2[:], m2)

            # fill m rows of the loss operands:
            # lh4 = [m1row; -1; m2row; -1], rh4 = [1|0; m1row|0; 0|1; 0|m2row]
            nc.sync.dma_start(lh4[0:1, :], m1)
            nc.scalar.dma_start(rh4[1:2, 0:S], m1)
            nc.sync.dma_start(lh4[2:3, :], m2)
            nc.scalar.dma_start(rh4[3:4, S:2 * S], m2)

            # ---- loss: sum_{j,k} ||m1_j-m1_k| - |m2_j-m2_k|| ----
            # one K=4 matmul per j-block -> [128, 512] = [d1 | d2]
            total = small.tile([128, 1], F32, tag="total")
            for jb in range(2):
                dc_ps = dps.tile([128, 2 * S], F32, tag="dcomb")
                nc.tensor.matmul(dc_ps[:], lh4[:, jb * 128:(jb + 1) * 128],
                                 rh4[:])
                absd = tailp.tile([128, 2 * S], F32, tag="absd")
                nc.scalar.activation(absd[:], dc_ps[:],
                                     mybir.ActivationFunctionType.Abs)
                dd = tailp.tile([128, S], F32, tag="dd")
                nc.vector.tensor_tensor(dd[:], absd[:, 0:S], absd[:, S:2 * S],
                                        op=mybir.AluOpType.subtract)
                part = small.tile([128, 1], F32, tag=f"part{jb}")
                nc.vector.tensor_reduce(
                    part[:], dd[:], axis=mybir.AxisListType.X,
                    op=mybir.AluOpType.add, apply_absolute_value=True)
                if jb == 0:
                    tot0 = part
                else:
                    nc.vector.tensor_add(total[:], tot0[:], part[:])

            loss_ps = ps1.tile([1, 1], F32, tag="t1ps")
            nc.tensor.matmul(loss_ps[:], ones128_sb[:], total[:])
            loss_sb = small.tile([1, 1], F32, tag="losssb")
            nc.vector.tensor_copy(loss_sb[:], loss_ps[:])
            nc.sync.dma_start(out[:], loss_sb[:])

    nc.compile()
    return nc


def _get_nc():
    if "nc" not in _CACHE:
        _CACHE["nc"] = _build_nc()
    return _CACHE["nc"]


def _host_inputs(input, feature, sp):
    sp32 = np.asarray(sp).astype(np.int32).reshape(B, H, W)
    lo = (sp32 & 7).astype(ml_dtypes.bfloat16)
    ohhi = ((sp32 >> 3)[..., None] ==
            np.arange(32, dtype=np.int32)).astype(ml_dtypes.float8_e4m3)
    ohhi = np.ascontiguousarray(ohhi.reshape(B, H, W * 32))
    iota16 = np.broadcast_to(
        np.arange(16, dtype=np.float32)[None, :], (128, 16)
    ).astype(ml_dtypes.bfloat16)
    iota16 = np.ascontiguousarray(iota16)
    ryt = np.ascontiguousarray(_interp_matrix(H, FH).T)   # [64, 256]
    rxt = np.ascontiguousarray(_interp_matrix(W, FW).T)   # [64, 256]
    ones64 = np.full((C, 1), 1.0 / C, dtype=ml_dtypes.bfloat16)
    ones128 = np.ones((128, 1), dtype=np.float32)
    onesrow = np.ones((1, S), dtype=np.float32)
    emat = np.zeros((128, 32), dtype=np.float32)
    for i in range(4):
        for jh in range(32):
            emat[32 * i + jh, jh] = 1.0
    mask = np.zeros((128, 96), dtype=np.float32)
    for i in range(4):
        mask[32 * i:32 * (i + 1), 24 * i:24 * (i + 1)] = 1.0
        mask[32 * i:32 * (i + 1), 24 * i:24 * i + 8] = 1.0 / C
    lh4b = np.zeros((4, 256), dtype=np.float32)
    lh4b[1, :] = -1.0
    lh4b[3, :] = -1.0
    rh4b = np.zeros((4, 512), dtype=np.float32)
    rh4b[0, 0:256] = 1.0
    rh4b[2, 256:512] = 1.0
    xf = np.ascontiguousarray(np.asarray(input, dtype=np.float32))
    ff = np.ascontiguousarray(
        np.asarray(feature, dtype=np.float32)
        .reshape(B, C, FH * FW).astype(ml_dtypes.bfloat16))
    in_maps = []
    for b in range(B):
        in_maps.append({
            "x": xf[b],
            "f": ff[b],
            "lo": np.ascontiguousarray(lo[b]),
            "ohhi": ohhi[b],
            "iota16": iota16,
            "ryt": ryt,
            "rxt": rxt,
            "ones64": ones64,
            "ones128": ones128,
            "onesrow": onesrow,
            "emat": emat,
            "mask": mask,
            "lh4b": lh4b,
            "rh4b": rh4b,
        })
    return in_maps


def _run(inputs, trace=False, **kw):
    nc = _get_nc()
    in_maps = _host_inputs(inputs["input"], inputs["feature"], inputs["sp"])
    res = run_bass_kernel_spmd(nc, in_maps, core_ids=list(range(N_CORES)),
                               trace=trace, **kw)
    sums = np.array([res.results[i]["out"][0, 0] for i in range(N_CORES)],
                    dtype=np.float64)
    loss = (sums / float(S * S)).mean()
    return np.float32(loss), res


def kernel(**inputs) -> np.ndarray:
    loss, _ = _run(inputs, trace=False)
    return np.asarray(loss, dtype=np.float32)


# revision 35
# speedup vs baseline: 1.0402x; 1.0176x over previous
"""ConsistencyLoss kernel for 8 TRN2 NeuronCores (Bass/Tile).

loss = mean_b mean_{j,k} | |m1_j - m1_k| - |m2_j - m2_k| |
  m1 = per-segment means of channel-mean(input)       [B,64,256,256]
  m2 = per-segment means of channel-mean(bilinear_up(feature)) [B,64,64,64]

Sharding: data-parallel over batch B=8, one batch element per core; the 8
per-core loss sums are averaged on the host (the "all-reduce" of the hint).

Per-core pipeline:
  - input channel-sum on DVE: 16-channel DMA chunks ([128h, 16c, 256w],
    1 KiB descriptors, h on partitions) reduced by a bf16 in-place add tree
    (level 1 fp32->bf16, then 2x-mode bf16 adds)
  - feature path: channel-mean via ones/64-stationary matmuls, separable
    align_corners bilinear upsample as two small matmuls with host-built
    interpolation matrices (T1 = fm @ RyT, fmup = T1^T-slices @ RxT)
  - segment reduction (S=256 = 32hi x 8lo one-hot decomposition):
    oh_hi one-hots ship from host as fp8 (exact 0/1); oh_lo + A1/A2 built
    on DVE in (w, j) layout; grouped cross-matrix matmuls - 4 pixel-columns
    share one [128,128] fp8 stationary, one N=96 bf16 moving block
    [A1|A2|oh_lo]x4 accumulates into a [128,96] PSUM tile across 128 groups;
    diagonal [32,24] blocks are extracted via host-mask multiply + E^T
    matmul + one reduce (mask also folds in the 1/64 channel scale)
  - counts come from the oh_lo columns; m = sum * recip(max(cnt, .5))
    reproduces the reference's where(cnt>0, sum/max(cnt,1), 0)
  - similarity L1: one K=4 outer-product matmul per 128-row block yields
    [d1|d2] = [m_j - m_k] for both m's; ACT abs, DVE subtract and
    abs-reduce, final ones-matmul -> scalar sum per core
Host prep (cheap O(HW) int ops, part of sharding): sp -> bf16 lo plane and
fp8 hi one-hot, interpolation matrices, small constant tables.
"""

import sys

if "/opt/trn_rl_repo" not in sys.path:
    sys.path.insert(0, "/opt/trn_rl_repo")

import numpy as np
import ml_dtypes

import concourse.bacc as bacc
import concourse.mybir as mybir
import concourse.tile as tile
from concourse.bass_utils import run_bass_kernel_spmd

B, C, H, W = 8, 64, 256, 256
FH, FW = 64, 64
S = 256
N_CORES = 8

F32 = mybir.dt.float32
BF16 = mybir.dt.bfloat16
FP8 = mybir.dt.float8e4

_CACHE = {}


def _interp_matrix(out_size: int, in_size: int) -> np.ndarray:
    """R [out,in]: bilinear align_corners row-interp matrix (float32)."""
    r = np.zeros((out_size, in_size), dtype=np.float64)
    ys = np.linspace(0.0, in_size - 1.0, out_size)
    y0 = np.floor(ys).astype(np.int64)
    y1 = np.minimum(y0 + 1, in_size - 1)
    wy = ys - y0
    for o in range(out_size):
        r[o, y0[o]] += 1.0 - wy[o]
        r[o, y1[o]] += wy[o]
    return r.astype(np.float32)


def _build_nc(dbg=False):
    nc = bacc.Bacc("TRN2", target_bir_lowering=False, debug=False,
                   num_devices=N_CORES)

    LOW = 8
    HIW = 32
    MOVW = 3 * LOW
    GW = 128 // HIW
    NG = W // GW

    x = nc.dram_tensor("x", [C, H, W], F32, kind="ExternalInput").ap()
    f = nc.dram_tensor("f", [C, FH * FW], BF16, kind="ExternalInput").ap()
    lo = nc.dram_tensor("lo", [H, W], BF16, kind="ExternalInput").ap()
    ohhi_in = nc.dram_tensor("ohhi", [H, W * 32], FP8, kind="ExternalInput").ap()
    iota16 = nc.dram_tensor("iota16", [128, 16], BF16, kind="ExternalInput").ap()
    ryt = nc.dram_tensor("ryt", [FH, H], F32, kind="ExternalInput").ap()
    rxt = nc.dram_tensor("rxt", [FW, W], F32, kind="ExternalInput").ap()
    ones64 = nc.dram_tensor("ones64", [C, 1], BF16, kind="ExternalInput").ap()
    ones128 = nc.dram_tensor("ones128", [128, 1], F32, kind="ExternalInput").ap()
    onesrow = nc.dram_tensor("onesrow", [1, S], F32, kind="ExternalInput").ap()
    emat = nc.dram_tensor("emat", [128, 32], F32, kind="ExternalInput").ap()
    lh4b = nc.dram_tensor("lh4b", [4, S], BF16, kind="ExternalInput").ap()
    rh4b = nc.dram_tensor("rh4b", [4, 2 * S], BF16, kind="ExternalInput").ap()
    mask = nc.dram_tensor("mask", [128, GW * MOVW], F32, kind="ExternalInput").ap()
    out = nc.dram_tensor("out", [1, 1], F32, kind="ExternalOutput").ap()
    if dbg:
        dbg_acc = nc.dram_tensor("dbg_acc", [32, 24], F32, kind="ExternalOutput").ap()
        dbg_px = nc.dram_tensor("dbg_px", [128, W], F32, kind="ExternalOutput").ap()
        dbg_m1 = nc.dram_tensor("dbg_m1", [32, 8], F32, kind="ExternalOutput").ap()
        dbg_m2 = nc.dram_tensor("dbg_m2", [32, 8], F32, kind="ExternalOutput").ap()

    CCH = 16          # channels per input DMA chunk
    NCC = C // CCH

    with tile.TileContext(nc) as tc:
        with (
            tc.tile_pool(name="const", bufs=1) as const,
            tc.tile_pool(name="xin", bufs=8) as xin,
            tc.tile_pool(name="tree", bufs=1) as treep,
            tc.tile_pool(name="mov", bufs=2) as movp,
            tc.tile_pool(name="work", bufs=2) as work,
            tc.tile_pool(name="small", bufs=2) as small,
            tc.tile_pool(name="tail", bufs=1) as tailp,
            tc.tile_pool(name="ps1", bufs=1, space="PSUM") as ps1,
            tc.tile_pool(name="fmp", bufs=2, space="PSUM") as fmp,
            tc.tile_pool(name="psacc", bufs=1, space="PSUM") as psacc,
            tc.tile_pool(name="dps", bufs=1, space="PSUM") as dps,
        ):
            # ---- constants (SWDGE path; sync queue reserved for x) ----
            iota_sb = const.tile([128, 16], BF16, tag="iota")
            nc.gpsimd.dma_start(iota_sb[:], iota16[:])
            ryt_sb = const.tile([FH, H], F32, tag="ryt")
            nc.gpsimd.dma_start(ryt_sb[:], ryt[:])
            rxt_sb = const.tile([FW, W], F32, tag="rxt")
            nc.gpsimd.dma_start(rxt_sb[:], rxt[:])
            ones64_sb = const.tile([C, 1], BF16, tag="o64")
            nc.gpsimd.dma_start(ones64_sb[:], ones64[:])
            ones128_sb = const.tile([128, 1], F32, tag="o128")
            nc.gpsimd.dma_start(ones128_sb[:], ones128[:])
            emat_sb = const.tile([128, 32], F32, tag="emat")
            nc.gpsimd.dma_start(emat_sb[:], emat[:])
            mask_sb = const.tile([128, GW * MOVW], F32, tag="mask")
            nc.gpsimd.dma_start(mask_sb[:], mask[:])

            # loss-stage operand bases prefilled early
            lh4 = const.tile([4, S], BF16, tag="lh4")
            nc.gpsimd.dma_start(lh4[:], lh4b[:])
            rh4 = const.tile([4, 2 * S], BF16, tag="rh4")
            nc.gpsimd.dma_start(rh4[:], rh4b[:])

            # ---- feature path: channel mean -> fm [64 h', 64 w'] ----
            fsb = const.tile([C, FH * FW], BF16, tag="fsb")
            nc.gpsimd.dma_start(fsb[:], f[:])
            fmsb = const.tile([FH, FW], F32, tag="fmsb")
            for i in range(8):
                fm_ps = fmp.tile([1, 512], F32, tag="fmps")
                nc.tensor.matmul(fm_ps[:], ones64_sb[:], fsb[:, i * 512:(i + 1) * 512])
                fmpart = small.tile([1, 512], F32, tag="fmpart")
                nc.scalar.copy(fmpart[:], fm_ps[:])
                nc.sync.dma_start(fmsb[i * 8:(i + 1) * 8, :], fmpart[:])

            # ---- bilinear upsample: fmup = Ry @ fm @ Rx^T ----
            t1_ps = ps1.tile([FW, H], F32, tag="t1ps")
            nc.tensor.matmul(t1_ps[:], fmsb[:], ryt_sb[:])
            t1_sb = const.tile([FW, H], F32, tag="t1sb")
            nc.scalar.copy(t1_sb[:], t1_ps[:])

            px2bf = []
            for hb in range(2):
                up_ps = ps1.tile([128, W], F32, tag="upps")
                nc.tensor.matmul(up_ps[:], t1_sb[:, hb * 128:(hb + 1) * 128],
                                 rxt_sb[:])
                p2 = work.tile([128, W], BF16, tag=f"px2bf{hb}")
                nc.scalar.copy(p2[:], up_ps[:])
                px2bf.append(p2)

            # ---- main loop over h-blocks ----
            lo_sbs, ohhi_sbs = [], []
            for hb in range(2):
                lo_t = const.tile([128, W], BF16, tag=f"losb{hb}")
                lo_sbs.append(lo_t)
                oh_t = const.tile([128, W, HIW], FP8, tag=f"ohhi{hb}")
                ohhi_sbs.append(oh_t)

            def _issue_id_dmas():
                for hb in range(2):
                    nc.sync.dma_start(lo_sbs[hb][:],
                                      lo[hb * 128:(hb + 1) * 128, :])
                    nc.sync.dma_start(
                        ohhi_sbs[hb].rearrange("p w j -> p (w j)"),
                        ohhi_in[hb * 128:(hb + 1) * 128, :])
            acc_ps = psacc.tile([128, GW * MOVW], F32, tag="acc")
            for hb in range(2):
                # input channel-sum via accumulate-DMA (CCE adds in DMA path):
                # 2 tiles x 4 overlaid 8-channel slabs -> [128, 8, 256] each,
                # then a small bf16 tree on DVE.
                parts = []
                for cc in range(NCC):
                    if hb == 0 and cc == 2:
                        _issue_id_dmas()
                    xt = xin.tile([128, CCH, W], F32, tag="xt")
                    nc.sync.dma_start(
                        xt[:],
                        x[cc * CCH:(cc + 1) * CCH,
                          hb * 128:(hb + 1) * 128, :].rearrange("c h w -> h c w"),
                    )
                    t8 = treep.tile([128, 8, W], BF16, tag=f"t8_{cc % 2}")
                    nc.vector.tensor_add(t8[:], xt[:, 0:8, :], xt[:, 8:16, :])
                    nc.vector.tensor_add(t8[:, 0:4, :], t8[:, 0:4, :],
                                         t8[:, 4:8, :])
                    nc.vector.tensor_add(t8[:, 0:2, :], t8[:, 0:2, :],
                                         t8[:, 2:4, :])
                    nc.vector.tensor_add(t8[:, 0:1, :], t8[:, 0:1, :],
                                         t8[:, 1:2, :])
                    if cc % 2 == 1:
                        psum_t = treep.tile([128, W], BF16, tag=f"pp{cc // 2}")
                        nc.vector.tensor_add(psum_t[:].unsqueeze(1),
                                             parts[-1][:, 0:1, :],
                                             t8[:, 0:1, :])
                        parts[-1] = psum_t
                    else:
                        parts.append(t8)
                px1 = work.tile([128, W], BF16, tag="px1bf")
                nc.vector.tensor_add(px1[:], parts[0][:], parts[1][:])
                if dbg and hb == 0:
                    pxf = tailp.tile([128, W], F32, tag="dbgpx")
                    nc.vector.tensor_copy(pxf[:], px1[:])
                    nc.sync.dma_start(dbg_px[:], pxf[:])

                lo_sb = lo_sbs[hb]
                ohhi = ohhi_sbs[hb]

                # lo one-hot + A tiles in (w, j) layout, built per w-half so
                # the grouped matmuls overlap the builds
                mov = movp.tile([128, W, MOVW], BF16, tag="mov")
                WH = W // 4
                # ohlo + A2 do not depend on x -> build during the stream
                for wh in range(4):
                    ws = slice(wh * WH, (wh + 1) * WH)
                    iota_b = iota_sb[:, 0:LOW].unsqueeze(1).to_broadcast(
                        [128, WH, LOW])
                    lo_b = lo_sb[:, ws].unsqueeze(2).to_broadcast(
                        [128, WH, LOW])
                    px2_b = px2bf[hb][:, ws].unsqueeze(2).to_broadcast(
                        [128, WH, LOW])
                    nc.vector.tensor_tensor(mov[:, ws, 2 * LOW:3 * LOW],
                                            iota_b, lo_b,
                                            op=mybir.AluOpType.is_equal)
                    nc.vector.tensor_tensor(mov[:, ws, LOW:2 * LOW],
                                            mov[:, ws, 2 * LOW:3 * LOW], px2_b,
                                            op=mybir.AluOpType.mult)
                # A1 needs px1; interleave with the grouped matmuls
                for wh in range(4):
                    ws = slice(wh * WH, (wh + 1) * WH)
                    px1_b = px1[:, ws].unsqueeze(2).to_broadcast(
                        [128, WH, LOW])
                    nc.vector.tensor_tensor(mov[:, ws, 0:LOW],
                                            mov[:, ws, 2 * LOW:3 * LOW], px1_b,
                                            op=mybir.AluOpType.mult)
                    ng2 = NG // 4
                    for g in range(wh * ng2, (wh + 1) * ng2):
                        lhs = ohhi[:, g * GW:(g + 1) * GW, :].rearrange(
                            "p w j -> p (w j)")
                        rhs = mov[:, g * GW:(g + 1) * GW, :].rearrange(
                            "p w j -> p (w j)")
                        nc.tensor.matmul(
                            acc_ps[:], lhs, rhs,
                            start=(hb == 0 and g == 0),
                            stop=(hb == 1 and g == NG - 1))

            # ---- diagonal extraction: mask off-diag blocks, sum row-blocks
            #      via E^T matmul, reduce slots on free axis ----
            acc_all = tailp.tile([128, GW * MOVW], F32, tag="accall")
            nc.vector.tensor_tensor(acc_all[:], acc_ps[:], mask_sb[:],
                                    op=mybir.AluOpType.mult)
            ex_ps = ps1.tile([HIW, GW * MOVW], F32, tag="expps")
            nc.tensor.matmul(ex_ps[:], emat_sb[:], acc_all[:])
            acc_sb = small.tile([HIW, MOVW], F32, tag="accsb")
            nc.vector.tensor_reduce(
                acc_sb[:], ex_ps.rearrange("p (s j) -> p j s", s=GW),
                axis=mybir.AxisListType.X, op=mybir.AluOpType.add)
            if dbg:
                nc.sync.dma_start(dbg_acc[:], acc_sb[:])

            # ---- m1/m2 [32, 8] -> combined [32, 16] ----
            cntm = small.tile([HIW, LOW], F32, tag="cntm")
            nc.vector.tensor_scalar_max(cntm[:], acc_sb[:, 2 * LOW:3 * LOW], 0.5)
            rc2 = small.tile([HIW, LOW], F32, tag="rc2")
            nc.vector.reciprocal(rc2[:], cntm[:])
            # sum1 columns of mask are pre-scaled by 1/C, so one shared rc works
            mcomb = small.tile([HIW, 2 * LOW], BF16, tag="mcomb")
            m1 = mcomb[:, 0:LOW]
            m2 = mcomb[:, LOW:2 * LOW]
            rc_b = rc2[:, :].unsqueeze(1).to_broadcast([HIW, 2, LOW])
            nc.vector.tensor_tensor(
                mcomb.rearrange("p (a b) -> p a b", a=2),
                acc_sb[:, 0:2 * LOW].rearrange("p (a b) -> p a b", a=2),
                rc_b, op=mybir.AluOpType.mult)
            if dbg:
                mdbg = small.tile([HIW, 2 * LOW], F32, tag="mdbg")
                nc.vector.tensor_copy(mdbg[:], mcomb[:])
                nc.sync.dma_start(dbg_m1[:], mdbg[:, 0:LOW])
                nc.sync.dma_start(dbg_m2[:], mdbg[:, LOW:2 * LOW])

            # fill m rows of the loss operands:
            # lh4 = [m1row; -1; m2row; -1], rh4 = [1|0; m1row|0; 0|1; 0|m2row]
            nc.sync.dma_start(lh4[0:1, :], m1)
            nc.scalar.dma_start(rh4[1:2, 0:S], m1)
            nc.sync.dma_start(lh4[2:3, :], m2)
            nc.scalar.dma_start(rh4[3:4, S:2 * S], m2)

            # ---- loss: sum_{j,k} ||m1_j-m1_k| - |m2_j-m2_k|| ----
            # one K=4 matmul per j-block -> [128, 512] = [d1 | d2]
            total = small.tile([128, 1], F32, tag="total")
            for jb in range(2):
                dc_ps = dps.tile([128, 2 * S], F32, tag="dcomb")
                nc.tensor.matmul(dc_ps[:], lh4[:, jb * 128:(jb + 1) * 128],
                                 rh4[:])
                absd = tailp.tile([128, 2 * S], F32, tag="absd")
                nc.scalar.activation(absd[:], dc_ps[:],
                                     mybir.ActivationFunctionType.Abs)
                dd = tailp.tile([128, S], F32, tag="dd")
                nc.vector.tensor_tensor(dd[:], absd[:, 0:S], absd[:, S:2 * S],
                                        op=mybir.AluOpType.subtract)
                part = small.tile([128, 1], F32, tag=f"part{jb}")
                nc.vector.tensor_reduce(
                    part[:], dd[:], axis=mybir.AxisListType.X,
                    op=mybir.AluOpType.add, apply_absolute_value=True)
                if jb == 0:
                    tot0 = part
                else:
                    nc.vector.tensor_add(total[:], tot0[:], part[:])

            loss_ps = ps1.tile([1, 1], F32, tag="t1ps")
            nc.tensor.matmul(loss_ps[:], ones128_sb[:], total[:])
            loss_sb = small.tile([1, 1], F32, tag="losssb")
            nc.vector.tensor_copy(loss_sb[:], loss_ps[:])
            nc.sync.dma_start(out[:], loss_sb[:])

    nc.compile()
    return nc


def _get_nc():
    if "nc" not in _CACHE:
        _CACHE["nc"] = _build_nc()
    return _CACHE["nc"]


def _host_inputs(input, feature, sp):
    sp32 = np.asarray(sp).astype(np.int32).reshape(B, H, W)
    lo = (sp32 & 7).astype(ml_dtypes.bfloat16)
    ohhi = ((sp32 >> 3)[..., None] ==
            np.arange(32, dtype=np.int32)).astype(ml_dtypes.float8_e4m3)
    ohhi = np.ascontiguousarray(ohhi.reshape(B, H, W * 32))
    iota16 = np.broadcast_to(
        np.arange(16, dtype=np.float32)[None, :], (128, 16)
    ).astype(ml_dtypes.bfloat16)
    iota16 = np.ascontiguousarray(iota16)
    ryt = np.ascontiguousarray(_interp_matrix(H, FH).T)   # [64, 256]
    rxt = np.ascontiguousarray(_interp_matrix(W, FW).T)   # [64, 256]
    ones64 = np.full((C, 1), 1.0 / C, dtype=ml_dtypes.bfloat16)
    ones128 = np.ones((128, 1), dtype=np.float32)
    onesrow = np.ones((1, S), dtype=np.float32)
    emat = np.zeros((128, 32), dtype=np.float32)
    for i in range(4):
        for jh in range(32):
            emat[32 * i + jh, jh] = 1.0
    mask = np.zeros((128, 96), dtype=np.float32)
    for i in range(4):
        mask[32 * i:32 * (i + 1), 24 * i:24 * (i + 1)] = 1.0
        mask[32 * i:32 * (i + 1), 24 * i:24 * i + 8] = 1.0 / C
    lh4b = np.zeros((4, 256), dtype=ml_dtypes.bfloat16)
    lh4b[1, :] = -1.0
    lh4b[3, :] = -1.0
    rh4b = np.zeros((4, 512), dtype=ml_dtypes.bfloat16)
    rh4b[0, 0:256] = 1.0
    rh4b[2, 256:512] = 1.0
    xf = np.ascontiguousarray(np.asarray(input, dtype=np.float32))
    ff = np.ascontiguousarray(
        np.asarray(feature, dtype=np.float32)
        .reshape(B, C, FH * FW).astype(ml_dtypes.bfloat16))
    in_maps = []
    for b in range(B):
        in_maps.append({
            "x": xf[b],
            "f": ff[b],
            "lo": np.ascontiguousarray(lo[b]),
            "ohhi": ohhi[b],
            "iota16": iota16,
            "ryt": ryt,
            "rxt": rxt,
            "ones64": ones64,
            "ones128": ones128,
            "onesrow": onesrow,
            "emat": emat,
            "mask": mask,
            "lh4b": lh4b,
            "rh4b": rh4b,
        })
    return in_maps


def _run(inputs, trace=False, **kw):
    nc = _get_nc()
    in_maps = _host_inputs(inputs["input"], inputs["feature"], inputs["sp"])
    res = run_bass_kernel_spmd(nc, in_maps, core_ids=list(range(N_CORES)),
                               trace=trace, **kw)
    sums = np.array([res.results[i]["out"][0, 0] for i in range(N_CORES)],
                    dtype=np.float64)
    loss = (sums / float(S * S)).mean()
    return np.float32(loss), res


def kernel(**inputs) -> np.ndarray:
    loss, _ = _run(inputs, trace=False)
    return np.asarray(loss, dtype=np.float32)
